# revision 1
# baseline (speedup 1.0000x reference)
"""2-layer GCN (PyG GCNConv semantics) on 8 Trainium2 NeuronCores.

Strategy (dst-sharded, per core):
  - Host: add self-loops, compute symmetric norm dinv = 1/sqrt(deg),
    sort each core's incoming edges by (window, src-half, sub-window),
    pad each (w,h,sw) group's slot count to a cross-core-uniform
    multiple of 128 so one SPMD program serves all 8 cores.
  - Device, per GCN layer:
      table[v] = dinv[v] * (h @ W)           (node-major bf16 rows, 256B stride)
      AllGather table shards
      per 512-node window: dma_gather 64B message rows (token slots),
      build one-hot S via DVE is_equal(nodeof, iota), PE matmuls
      psum[:, 64*sw:64*sw+64] += M_chunk.T @ S_chunk  (feature-major psum),
      evict: out = relu(dinv_dst * psum + b) on DVE+ACT.
  - Final: logits = Wout.T @ out2 (feature-major), host transposes.
"""

import sys
import numpy as np

sys.path.insert(0, "/opt/trn_rl_repo")

import ml_dtypes  # noqa: E402

BF16 = ml_dtypes.bfloat16


# ----------------------------------------------------------------------------
# Config
# ----------------------------------------------------------------------------
class Cfg:
    def __init__(self, N, E, F, HID, ACT, NC, WIN, SCOLS):
        self.N, self.E, self.F, self.HID, self.ACT, self.NC = N, E, F, HID, ACT, NC
        self.WIN, self.SCOLS = WIN, SCOLS
        self.NLOC = N // NC                       # real nodes per core
        assert self.NLOC * NC == N
        self.NWIN = -(-self.NLOC // WIN)          # windows per core
        self.NLOCP = self.NWIN * WIN              # padded nodes per core
        assert self.NLOCP % 128 == 0
        self.NSW = WIN // SCOLS                   # sub-windows per window
        self.TILES = self.NLOCP // 128            # 128-node tiles per core
        self.VROWS = NC * self.NLOCP              # padded global rows
        self.HALF = self.VROWS // 2               # rows per src half
        assert self.HALF <= 32767, "int16 gather index overflow"
        assert self.HALF % self.NLOCP == 0, "half boundary must align to core shards"
        self.TSTRIDE = 128                        # bf16 elements per table row (256B)


CFG_FULL = Cfg(N=50000, E=1600000, F=128, HID=32, ACT=64, NC=8, WIN=512, SCOLS=64)
# 50000/8 = 6250 -> NWIN=13, NLOCP=6656, VROWS=53248, HALF=26624


# ----------------------------------------------------------------------------
# Host preprocessing
# ----------------------------------------------------------------------------
def preprocess(x, edge_index, W1, b1, W2, b2, Wout, bout, cfg):
    """Returns (in_maps, meta). meta drives program structure and must be
    identical for every core (it is: NCH is maxed across cores)."""
    N, NC, NLOC, NLOCP = cfg.N, cfg.NC, cfg.NLOC, cfg.NLOCP
    NWIN, WIN, NSW, SCOLS = cfg.NWIN, cfg.WIN, cfg.NSW, cfg.SCOLS

    src = np.asarray(edge_index[0], dtype=np.int64)
    dst = np.asarray(edge_index[1], dtype=np.int64)
    loop = np.arange(N, dtype=np.int64)
    src = np.concatenate([src, loop])
    dst = np.concatenate([dst, loop])
    M = src.shape[0]

    deg = np.bincount(dst, minlength=N).astype(np.float64)
    dinv = np.where(deg > 0, 1.0 / np.sqrt(deg), 0.0).astype(np.float32)

    core = dst // NLOC
    local = dst - core * NLOC
    w = local // WIN
    sw = (local % WIN) // SCOLS
    nodeof = (local % SCOLS).astype(np.float32)

    csrc = src // NLOC
    vv = csrc * NLOCP + (src - csrc * NLOC)       # padded virtual row of src
    half = vv // cfg.HALF
    idxval = (vv - half * cfg.HALF).astype(np.int16)

    NKEY_PER_CORE = NWIN * 2 * NSW
    key = ((core * NWIN + w) * 2 + half) * NSW + sw
    cnt = np.bincount(key, minlength=NC * NKEY_PER_CORE).reshape(NC, NWIN, 2, NSW)
    NCH = -(-cnt // 128)                          # ceil
    NCH = NCH.max(axis=0)                         # [NWIN, 2, NSW] cross-core program
    slots_per_key = (NCH * 128).ravel()           # flat (w,h,sw)
    slot_base = np.concatenate([[0], np.cumsum(slots_per_key)])
    TOK = int(slot_base[-1])
    TOTCH = TOK // 128

    order = np.argsort(key, kind="stable")
    ks = key[order]
    gstart = np.concatenate([[0], np.cumsum(cnt.ravel())])[:-1]
    pos = np.arange(M) - gstart[ks]
    kk = ks % NKEY_PER_CORE
    slot = slot_base[kk] + pos
    oc = ks // NKEY_PER_CORE

    idxarr = np.zeros((NC, TOK), np.int16)        # pad slots gather row 0
    nodearr = np.full((NC, TOK), -1, np.float32)  # pad slots match no column
    idxarr[oc, slot] = idxval[order]
    nodearr[oc, slot] = nodeof[order]

    # per-(w,h) token ranges and chunk ranges
    tok_of = slot_base.reshape(-1)                # len NKEY+1 over (w,h,sw)
    meta = {
        "NCH": NCH,
        "TOK": TOK,
        "TOTCH": TOTCH,
        "tok_range": {},                          # (w,h) -> (tok0, tok1)
    }
    for wi in range(NWIN):
        for h in range(2):
            k0 = (wi * 2 + h) * NSW
            meta["tok_range"][(wi, h)] = (int(tok_of[k0]), int(tok_of[k0 + NSW]))

    # per-core inputs
    x = np.asarray(x, np.float32)
    dinv_pad = np.zeros(NC * NLOCP, np.float32)
    for c in range(NC):
        dinv_pad[c * NLOCP : c * NLOCP + NLOC] = dinv[c * NLOC : (c + 1) * NLOC]

    in_maps = []
    iota = np.tile(np.arange(SCOLS, dtype=np.float32)[None, :], (128, 1))
    for c in range(NC):
        xc = np.zeros((cfg.F, NLOCP), np.float32)
        xc[:, :NLOC] = x[c * NLOC : (c + 1) * NLOC].T
        dl = dinv_pad[c * NLOCP : (c + 1) * NLOCP]
        idx_rep = np.tile(idxarr[c].reshape(TOK // 16, 16).T, (8, 1))
        node_t = nodearr[c].reshape(TOTCH, 128).T
        in_maps.append(
            {
                "xT": xc.copy(),
                "W1f": np.asarray(W1, np.float32),
                "W2f": np.asarray(W2, np.float32),
                "Woutf": np.asarray(Wout, np.float32),
                "b1col": np.asarray(b1, np.float32).reshape(cfg.HID, 1),
                "b2col": np.asarray(b2, np.float32).reshape(cfg.HID, 1),
                "boutcol": np.asarray(bout, np.float32).reshape(cfg.ACT, 1),
                "dinvb": dl.reshape(cfg.TILES, 128).T.copy(),      # [128, TILES]
                "dinvrep": np.tile(dl[None, :], (cfg.HID, 1)),     # [HID, NLOCP]
                "idx16": np.ascontiguousarray(idx_rep),            # [128, TOK/16]
                "node16": np.ascontiguousarray(node_t),            # [128, TOTCH]
                "iota16": iota,                                    # [128, SCOLS]
            }
        )
    return in_maps, meta


# ----------------------------------------------------------------------------
# Device program
# ----------------------------------------------------------------------------
def emit_dma_gather(gp, out_ap, in_ap, idxs_ap, num_idxs, elem_size, elem_step):
    """bass.dma_gather (non-transpose, DRAM src) without the 256B elem assert
    (the 256B constraint applies to the row *stride*, encoded below)."""
    import concourse.mybir as mybir
    from concourse import ap_utils
    from concourse.bass import round_up_to_multiple, exact_div

    self = gp
    assert idxs_ap.dtype == mybir.dt.int16
    assert in_ap.dtype == out_ap.dtype
    assert ap_utils.ap_is_contiguous(in_ap.ap[1:])
    assert ap_utils.ap_is_contiguous(out_ap.ap[1:])
    assert ap_utils.ap_is_contiguous(idxs_ap.ap[1:])
    assert in_ap.ap[-1][1] == out_ap.ap[-1][1] == elem_size
    assert out_ap.ap[0][1] * out_ap.ap[1][1] == round_up_to_multiple(num_idxs, 128)
    assert in_ap.ap[0][0] == elem_step
    stride_bytes = elem_step * mybir.dt.size(in_ap.dtype)
    stride_bytes_256 = exact_div(stride_bytes, 256)
    assert stride_bytes_256 < 256
    _in_ap = self.lower_ap_dma(in_ap, for_custom_bir_dma=True)
    _idxs_ap = self.lower_ap(idxs_ap)
    _out_ap = self.lower_ap(out_ap)
    return self.add_instruction(
        mybir.InstDMAGatherAnt(
            name=self.bass.get_next_instruction_name(),
            ins=[*_in_ap, _idxs_ap, self.lower_val_access(self.to_reg(num_idxs))],
            outs=[_out_ap],
            transpose=False,
            num_idxs=num_idxs,
            elem_size=elem_size,
            stride_bytes_256=stride_bytes_256,
            gen_mode=0,
            single_packet=False,
            queue_num=0,
            sbuf_tokens_per_rank=0,
            sbuf_free_dim_per_rank=0,
            sbuf_free_dim_pad_per_rank=0,
            sbuf_byte_offset=0,
        )
    )


def build(meta, cfg, stage=99, sub=0):
    import concourse.mybir as mybir
    import concourse.tile as tile
    from concourse.bacc import Bacc
    from concourse import library_config
    from concourse.tile_rust import add_dep_helper

    f32, bf16, i16 = mybir.dt.float32, mybir.dt.bfloat16, mybir.dt.int16
    Alu = mybir.AluOpType
    Act = mybir.ActivationFunctionType
    NCH, TOK, TOTCH = meta["NCH"], meta["TOK"], meta["TOTCH"]
    NWIN, WIN, NSW, SCOLS = cfg.NWIN, cfg.WIN, cfg.NSW, cfg.SCOLS
    HID, F, ACTD, TILES = cfg.HID, cfg.F, cfg.ACT, cfg.TILES
    NLOCP, VROWS, HALF, TS = cfg.NLOCP, cfg.VROWS, cfg.HALF, cfg.TSTRIDE

    nc = Bacc("TRN2", target_bir_lowering=False, debug=False, num_devices=cfg.NC)

    # I/O
    xT = nc.dram_tensor("xT", [F, NLOCP], f32, kind="ExternalInput")
    W1f = nc.dram_tensor("W1f", [F, HID], f32, kind="ExternalInput")
    W2f = nc.dram_tensor("W2f", [HID, HID], f32, kind="ExternalInput")
    Woutf = nc.dram_tensor("Woutf", [HID, ACTD], f32, kind="ExternalInput")
    b1col = nc.dram_tensor("b1col", [HID, 1], f32, kind="ExternalInput")
    b2col = nc.dram_tensor("b2col", [HID, 1], f32, kind="ExternalInput")
    boutcol = nc.dram_tensor("boutcol", [ACTD, 1], f32, kind="ExternalInput")
    dinvb = nc.dram_tensor("dinvb", [128, TILES], f32, kind="ExternalInput")
    dinvrep = nc.dram_tensor("dinvrep", [HID, NLOCP], f32, kind="ExternalInput")
    idx16 = nc.dram_tensor("idx16", [128, TOK // 16], i16, kind="ExternalInput")
    node16 = nc.dram_tensor("node16", [128, TOTCH], f32, kind="ExternalInput")
    iota16 = nc.dram_tensor("iota16", [128, SCOLS], f32, kind="ExternalInput")
    out_fm = nc.dram_tensor("out_fm", [ACTD, NLOCP], f32, kind="ExternalOutput")
    tbl_dbg = nc.dram_tensor("tbl_dbg", [VROWS, TS], bf16, kind="ExternalInput") if sub == 7 else None

    # internal DRAM: fat gather tables (local shard + allgathered full)
    tbl_loc = [nc.dram_tensor(f"tbl_loc{l}", [NLOCP, TS], bf16) for l in (1, 2)]
    tbl_full = [nc.dram_tensor(f"tbl_full{l}", [VROWS, TS], bf16) for l in (1, 2)]

    with tile.TileContext(nc) as tc:
        nc.gpsimd.load_library(library_config.mlp)
        from contextlib import ExitStack

        with ExitStack() as ctx:
            consts = ctx.enter_context(tc.tile_pool(name="consts", bufs=1))
            mpool = ctx.enter_context(tc.tile_pool(name="msgs", bufs=4))
            spool = ctx.enter_context(tc.tile_pool(name="sel", bufs=2))
            ipool = ctx.enter_context(tc.tile_pool(name="idxt", bufs=3))
            wkpool = ctx.enter_context(tc.tile_pool(name="work", bufs=2))
            pagg = ctx.enter_context(tc.tile_pool(name="pagg", bufs=2, space="PSUM"))
            ptbl = ctx.enter_context(tc.tile_pool(name="ptbl", bufs=2, space="PSUM"))
            pout = ctx.enter_context(tc.tile_pool(name="pout", bufs=2, space="PSUM"))

            # ---- resident constants
            w1_t = consts.tile([F, HID], f32)
            nc.sync.dma_start(w1_t[:], W1f[:])
            w2_t = consts.tile([HID, HID], f32)
            nc.sync.dma_start(w2_t[:], W2f[:])
            wout_t = consts.tile([HID, ACTD], f32)
            nc.sync.dma_start(wout_t[:], Woutf[:])
            b1_t = consts.tile([HID, 1], f32)
            nc.sync.dma_start(b1_t[:], b1col[:])
            b2_t = consts.tile([HID, 1], f32)
            nc.sync.dma_start(b2_t[:], b2col[:])
            bout_t = consts.tile([ACTD, 1], f32)
            nc.sync.dma_start(bout_t[:], boutcol[:])
            dinvb_t = consts.tile([128, TILES], f32)
            nc.sync.dma_start(dinvb_t[:], dinvb[:])
            dinvrep_t = consts.tile([HID, NLOCP], f32)
            nc.sync.dma_start(dinvrep_t[:], dinvrep[:])
            iota_t = consts.tile([128, SCOLS], f32)
            nc.sync.dma_start(iota_t[:], iota16[:])
            node_t = consts.tile([128, TOTCH], f32)
            nc.sync.dma_start(node_t[:], node16[:])
            xT_t = consts.tile([F, NLOCP], f32)
            nc.sync.dma_start(xT_t[:], xT[:])
            out1_t = consts.tile([HID, NLOCP], f32)

            def build_table(l, src_fm, kdim, w_t):
                """table_l = dinv * (src_fm.T @ W) ; src_fm [kdim, NLOCP]."""
                if sub == 8:
                    return
                BATCH = 16
                nb = -(-TILES // BATCH)
                for b in range(nb):
                    t0, t1 = b * BATCH, min((b + 1) * BATCH, TILES)
                    ps = ptbl.tile([128, BATCH, HID], f32, tag="ptbl")
                    for t in range(t0, t1):
                        nc.tensor.matmul(
                            ps[:, t - t0, :],
                            src_fm[:, t * 128 : (t + 1) * 128],
                            w_t[:],
                            start=True,
                            stop=True,
                        )
                    sb = wkpool.tile([128, BATCH, HID], bf16, tag="tblsb")
                    nc.vector.tensor_tensor(
                        sb[:, : t1 - t0, :],
                        ps[:, : t1 - t0, :],
                        dinvb_t[:, t0:t1, None].to_broadcast([128, t1 - t0, HID]),
                        op=Alu.mult,
                    )
                    # rows 128t+p of the fat local table, payload cols 0:HID
                    dst = tbl_loc[l][:, 0:HID].rearrange(
                        "(t p) d -> p t d", p=128
                    )[:, t0:t1, :]
                    nc.sync.dma_start(dst, sb[:, : t1 - t0, :])
                # allgather shards into the full fat table
                nc.gpsimd.collective_compute(
                    "AllGather",
                    Alu.bypass,
                    replica_groups=[list(range(cfg.NC))],
                    ins=[tbl_loc[l].ap().opt()],
                    outs=[tbl_full[l].ap().opt()],
                )

            def aggregate(l, out_t, bias_t):
                """out_t[:, :] = relu(dinv_dst * (S.T-reduced gathered msgs) + b)."""
                tblF = tbl_dbg if sub == 7 else tbl_full[l]
                ch_cursor = 0
                last_sbuild = [None]
                last_gather = [None]
                for wi in range(NWIN):
                    ps = pagg.tile([HID, WIN], f32, tag="pagg")
                    wch0 = ch_cursor
                    # gather both halves' messages for this window
                    mtiles = {}
                    for h in (0, 1):
                        tok0, tok1 = meta["tok_range"][(wi, h)]
                        ntok = tok1 - tok0
                        if ntok == 0:
                            continue
                        mt = mpool.tile([128, ntok // 128, HID], bf16, tag=f"m{h}")
                        it = ipool.tile([128, ntok // 16], i16, tag=f"i{h}")
                        nc.sync.dma_start(it[:], idx16[:, tok0 // 16 : tok1 // 16])
                        if sub == 3:
                            nc.vector.memset(mt[:], 0.25)
                        if sub not in (2, 3):  # sub6 keeps gathers
                            g = emit_dma_gather(
                                nc.gpsimd,
                                mt[:],
                                tblF[h * HALF : (h + 1) * HALF, 0:HID],
                                it[:],
                                ntok,
                                HID,
                                TS,
                            )
                            if last_sbuild[0] is not None:
                                add_dep_helper(g.ins, last_sbuild[0], sync=True,
                                               reason="swdge-vs-dve2port")
                            last_gather[0] = g.ins
                        mtiles[h] = mt
                    # one-hot S for all chunks of this window
                    wch = sum(int(NCH[wi, h, s]) for h in (0, 1) for s in range(NSW))
                    if sub == 5 and wch > 0:
                        st = spool.tile([128, wch, SCOLS], bf16, tag="sel")
                        nc.vector.tensor_tensor(
                            st[:, 0, :], node_t[:, 0:1].to_broadcast([128, SCOLS]),
                            iota_t[:], op=Alu.is_equal)
                        nc.vector.memset(st[:, 1:, :], 0.0)
                    if wch > 0 and sub not in (2, 5, 6):
                        st = spool.tile([128, wch, SCOLS], bf16, tag="sel")
                        sb = nc.vector.tensor_tensor(
                            st[:],
                            node_t[:, wch0 : wch0 + wch, None].to_broadcast(
                                [128, wch, SCOLS]
                            ),
                            iota_t[:, None, :].to_broadcast([128, wch, SCOLS]),
                            op=Alu.is_equal,
                        )
                        if last_gather[0] is not None:
                            add_dep_helper(sb.ins, last_gather[0], sync=True,
                                           reason="dve2port-vs-swdge")
                        last_sbuild[0] = sb.ins
                    # matmul-accumulate
                    if sub in (1, 2, 6):
                        nc.vector.memset(ps[:], 0.5)
                    first = True
                    chw = 0
                    for h in (() if sub in (1, 2, 6) else (0, 1)):
                        mh = 0
                        for s in range(NSW):
                            for _k in range(int(NCH[wi, h, s])):
                                is_last = chw == wch - 1
                                nc.tensor.matmul(
                                    ps[:, s * SCOLS : (s + 1) * SCOLS],
                                    mtiles[h][:, mh, :],
                                    st[:, chw, :],
                                    start=first,
                                    stop=is_last,
                                )
                                first = False
                                mh += 1
                                chw += 1
                    ch_cursor += wch
                    # evict window: relu(dinv * psum + b)
                    tmp = wkpool.tile([HID, WIN], f32, tag="evt")
                    nc.vector.tensor_tensor(
                        tmp[:],
                        ps[:],
                        dinvrep_t[:, wi * WIN : (wi + 1) * WIN],
                        op=Alu.mult,
                    )
                    nc.scalar.activation(
                        out_t[:, wi * WIN : (wi + 1) * WIN], tmp[:], Act.Relu,
                        bias=bias_t[:],
                    )

            # ---- layer 1
            build_table(0, xT_t[:], F, w1_t)
            if stage >= 2:
                aggregate(0, out1_t, b1_t)
            else:
                nc.vector.memset(out1_t[:], 0.125)
            # ---- layer 2
            if stage >= 3:
                build_table(1, out1_t[:], HID, w2_t)
            out2_t = consts.tile([HID, NLOCP], f32)
            if stage >= 4:
                aggregate(1, out2_t, b2_t)
            else:
                nc.vector.memset(out2_t[:], 0.25)
            # ---- output layer: logits_fm = Wout.T @ out2 + bout
            for wi in range(NWIN):
                ps = pout.tile([ACTD, WIN], f32, tag="pl")
                nc.tensor.matmul(
                    ps[:],
                    wout_t[:],
                    out2_t[:, wi * WIN : (wi + 1) * WIN],
                    start=True,
                    stop=True,
                )
                lsb = wkpool.tile([ACTD, WIN], f32, tag="lsb")
                nc.vector.tensor_scalar_add(lsb[:], ps[:], bout_t[:])
                nc.sync.dma_start(out_fm[:, wi * WIN : (wi + 1) * WIN], lsb[:])

    nc.compile()
    return nc


# ----------------------------------------------------------------------------
# Entry point
# ----------------------------------------------------------------------------
_CACHE = {}


def run(x, edge_index, W1, b1, W2, b2, Wout, bout, cfg, trace=False):
    from concourse import bass_utils

    in_maps, meta = preprocess(x, edge_index, W1, b1, W2, b2, Wout, bout, cfg)
    key = (cfg.N, cfg.E, meta["TOK"], tuple(meta["NCH"].ravel().tolist()))
    import os
    stage = int(os.environ.get('K_STAGE', '99'))
    sub = int(os.environ.get('K_SUB', '0'))
    key = key + (stage, sub)
    if key not in _CACHE:
        _CACHE[key] = build(meta, cfg, stage, sub)
    nc = _CACHE[key]
    res = bass_utils.run_bass_kernel_spmd(
        nc, in_maps, core_ids=list(range(cfg.NC)), trace=trace
    )
    out = np.empty((cfg.N, cfg.ACT), np.float32)
    for c in range(cfg.NC):
        out[c * cfg.NLOC : (c + 1) * cfg.NLOC] = (
            res.results[c]["out_fm"][:, : cfg.NLOC].T
        )
    return out, res


def kernel(x, edge_index, W1, b1, W2, b2, Wout, bout):
    out, _ = run(x, edge_index, W1, b1, W2, b2, Wout, bout, CFG_FULL)
    return out



# revision 8
# speedup vs baseline: 2.6507x; 2.6507x over previous
"""2-layer GCN (PyG GCNConv semantics) on 8 Trainium2 NeuronCores.

Strategy (dst-sharded, per core):
  - Host: add self-loops, compute symmetric norm dinv = 1/sqrt(deg),
    sort each core's incoming edges by (window, src-half, sub-window),
    pad each (w,h,sw) group's slot count to a cross-core-uniform
    multiple of 128 so one SPMD program serves all 8 cores.
  - Device, per GCN layer:
      table[v] = dinv[v] * (h @ W)           (node-major bf16 rows, 256B stride)
      AllGather table shards
      per 512-node window: dma_gather 64B message rows (token slots),
      build one-hot S via DVE is_equal(nodeof, iota), PE matmuls
      psum[:, 64*sw:64*sw+64] += M_chunk.T @ S_chunk  (feature-major psum),
      evict: out = relu(dinv_dst * psum + b) on DVE+ACT.
  - Final: logits = Wout.T @ out2 (feature-major), host transposes.
"""

import sys
import numpy as np

sys.path.insert(0, "/opt/trn_rl_repo")

import ml_dtypes  # noqa: E402

BF16 = ml_dtypes.bfloat16


# ----------------------------------------------------------------------------
# Config
# ----------------------------------------------------------------------------
class Cfg:
    def __init__(self, N, E, F, HID, ACT, NC, WIN, SCOLS):
        self.N, self.E, self.F, self.HID, self.ACT, self.NC = N, E, F, HID, ACT, NC
        self.WIN, self.SCOLS = WIN, SCOLS
        self.NLOC = N // NC                       # real nodes per core
        assert self.NLOC * NC == N
        self.NWIN = -(-self.NLOC // WIN)          # windows per core
        self.NLOCP = self.NWIN * WIN              # padded nodes per core
        assert self.NLOCP % 128 == 0
        self.NSW = WIN // SCOLS                   # sub-windows per window
        self.TILES = self.NLOCP // 128            # 128-node tiles per core
        self.VROWS = NC * self.NLOCP              # padded global rows
        self.HALF = self.VROWS // 2               # rows per src half
        assert self.HALF <= 32767, "int16 gather index overflow"
        assert self.HALF % self.NLOCP == 0, "half boundary must align to core shards"
        self.TSTRIDE = 128                        # bf16 elements per table row (256B)


CFG_FULL = Cfg(N=50000, E=1600000, F=128, HID=32, ACT=64, NC=8, WIN=512, SCOLS=64)
# 50000/8 = 6250 -> NWIN=13, NLOCP=6656, VROWS=53248, HALF=26624


# ----------------------------------------------------------------------------
# Host preprocessing
# ----------------------------------------------------------------------------
def preprocess(x, edge_index, W1, b1, W2, b2, Wout, bout, cfg):
    """Returns (in_maps, meta). meta drives program structure and must be
    identical for every core (it is: NCH is maxed across cores)."""
    N, NC, NLOC, NLOCP = cfg.N, cfg.NC, cfg.NLOC, cfg.NLOCP
    NWIN, WIN, NSW, SCOLS = cfg.NWIN, cfg.WIN, cfg.NSW, cfg.SCOLS

    src = np.asarray(edge_index[0], dtype=np.int64)
    dst = np.asarray(edge_index[1], dtype=np.int64)
    loop = np.arange(N, dtype=np.int64)
    src = np.concatenate([src, loop])
    dst = np.concatenate([dst, loop])
    M = src.shape[0]

    deg = np.bincount(dst, minlength=N).astype(np.float64)
    dinv = np.where(deg > 0, 1.0 / np.sqrt(deg), 0.0).astype(np.float32)

    core = dst // NLOC
    local = dst - core * NLOC
    w = local // WIN
    sw = (local % WIN) // SCOLS
    nodeof = (local % SCOLS).astype(np.float32)

    csrc = src // NLOC
    vv = csrc * NLOCP + (src - csrc * NLOC)       # padded virtual row of src
    half = vv // cfg.HALF
    idxval = (vv - half * cfg.HALF).astype(np.int16)

    NKEY_PER_CORE = NWIN * 2 * NSW
    key = ((core * NWIN + w) * 2 + half) * NSW + sw
    cnt = np.bincount(key, minlength=NC * NKEY_PER_CORE).reshape(NC, NWIN, 2, NSW)
    NCH = -(-cnt // 128)                          # ceil
    NCH = NCH.max(axis=0)                         # [NWIN, 2, NSW] cross-core program
    slots_per_key = (NCH * 128).ravel()           # flat (w,h,sw)
    slot_base = np.concatenate([[0], np.cumsum(slots_per_key)])
    TOK = int(slot_base[-1])
    TOTCH = TOK // 128

    order = np.argsort(key, kind="stable")
    ks = key[order]
    gstart = np.concatenate([[0], np.cumsum(cnt.ravel())])[:-1]
    pos = np.arange(M) - gstart[ks]
    kk = ks % NKEY_PER_CORE
    slot = slot_base[kk] + pos
    oc = ks // NKEY_PER_CORE

    idxarr = np.zeros((NC, TOK), np.int16)        # pad slots gather row 0
    nodearr = np.full((NC, TOK), -1, np.float32)  # pad slots match no column
    idxarr[oc, slot] = idxval[order]
    nodearr[oc, slot] = nodeof[order]

    # per-(w,h) token ranges and chunk ranges
    tok_of = slot_base.reshape(-1)                # len NKEY+1 over (w,h,sw)
    meta = {
        "NCH": NCH,
        "TOK": TOK,
        "TOTCH": TOTCH,
        "tok_range": {},                          # (w,h) -> (tok0, tok1)
    }
    for wi in range(NWIN):
        for h in range(2):
            k0 = (wi * 2 + h) * NSW
            meta["tok_range"][(wi, h)] = (int(tok_of[k0]), int(tok_of[k0 + NSW]))

    # per-core inputs
    x = np.asarray(x, np.float32)
    dinv_pad = np.zeros(NC * NLOCP, np.float32)
    for c in range(NC):
        dinv_pad[c * NLOCP : c * NLOCP + NLOC] = dinv[c * NLOC : (c + 1) * NLOC]

    in_maps = []
    iota = np.tile(np.arange(SCOLS, dtype=np.float32)[None, :], (128, 1))
    for c in range(NC):
        xc = np.zeros((cfg.F, NLOCP), np.float32)
        xc[:, :NLOC] = x[c * NLOC : (c + 1) * NLOC].T
        dl = dinv_pad[c * NLOCP : (c + 1) * NLOCP]
        idx_rep = np.tile(idxarr[c].reshape(TOK // 16, 16).T, (8, 1))
        node_t = nodearr[c].reshape(TOTCH, 128).T
        in_maps.append(
            {
                "xT": xc.copy(),
                "W1f": np.asarray(W1, np.float32),
                "W2f": np.asarray(W2, np.float32),
                "Woutf": np.asarray(Wout, np.float32),
                "b1col": np.asarray(b1, np.float32).reshape(cfg.HID, 1),
                "b2col": np.asarray(b2, np.float32).reshape(cfg.HID, 1),
                "boutcol": np.asarray(bout, np.float32).reshape(cfg.ACT, 1),
                "dinvb": dl.reshape(cfg.TILES, 128).T.copy(),      # [128, TILES]
                "dinvrep": np.tile(dl[None, :], (cfg.HID, 1)),     # [HID, NLOCP]
                "idx16": np.ascontiguousarray(idx_rep),            # [128, TOK/16]
                "node16": np.ascontiguousarray(node_t),            # [128, TOTCH]
                "iota16": iota,                                    # [128, SCOLS]
            }
        )
    return in_maps, meta


# ----------------------------------------------------------------------------
# Device program
# ----------------------------------------------------------------------------
def emit_dma_gather(gp, out_ap, in_ap, idxs_ap, num_idxs, elem_size, elem_step,
                    queue_num=0):
    """bass.dma_gather (non-transpose, DRAM src) without the 256B elem assert
    (the 256B constraint applies to the row *stride*, encoded below)."""
    import concourse.mybir as mybir
    from concourse import ap_utils
    from concourse.bass import round_up_to_multiple, exact_div

    self = gp
    assert idxs_ap.dtype == mybir.dt.int16
    assert in_ap.dtype == out_ap.dtype
    assert ap_utils.ap_is_contiguous(in_ap.ap[1:])
    assert ap_utils.ap_is_contiguous(out_ap.ap[1:])
    assert ap_utils.ap_is_contiguous(idxs_ap.ap[1:])
    assert in_ap.ap[-1][1] == out_ap.ap[-1][1] == elem_size
    assert out_ap.ap[0][1] * out_ap.ap[1][1] == round_up_to_multiple(num_idxs, 128)
    assert in_ap.ap[0][0] == elem_step
    stride_bytes = elem_step * mybir.dt.size(in_ap.dtype)
    stride_bytes_256 = exact_div(stride_bytes, 256)
    assert stride_bytes_256 < 256
    _in_ap = self.lower_ap_dma(in_ap, for_custom_bir_dma=True)
    _idxs_ap = self.lower_ap(idxs_ap)
    _out_ap = self.lower_ap(out_ap)
    return self.add_instruction(
        mybir.InstDMAGatherAnt(
            name=self.bass.get_next_instruction_name(),
            ins=[*_in_ap, _idxs_ap, self.lower_val_access(self.to_reg(num_idxs))],
            outs=[_out_ap],
            transpose=False,
            num_idxs=num_idxs,
            elem_size=elem_size,
            stride_bytes_256=stride_bytes_256,
            gen_mode=0,
            single_packet=False,
            queue_num=queue_num,
            sbuf_tokens_per_rank=0,
            sbuf_free_dim_per_rank=0,
            sbuf_free_dim_pad_per_rank=0,
            sbuf_byte_offset=0,
        )
    )


def build(meta, cfg, stage=99, sub=0):
    import concourse.mybir as mybir
    import concourse.tile as tile
    from concourse.bacc import Bacc
    from concourse import library_config
    from concourse.tile_rust import add_dep_helper

    f32, bf16, i16 = mybir.dt.float32, mybir.dt.bfloat16, mybir.dt.int16
    Alu = mybir.AluOpType
    Act = mybir.ActivationFunctionType
    NCH, TOK, TOTCH = meta["NCH"], meta["TOK"], meta["TOTCH"]
    NWIN, WIN, NSW, SCOLS = cfg.NWIN, cfg.WIN, cfg.NSW, cfg.SCOLS
    HID, F, ACTD, TILES = cfg.HID, cfg.F, cfg.ACT, cfg.TILES
    NLOCP, VROWS, HALF, TS = cfg.NLOCP, cfg.VROWS, cfg.HALF, cfg.TSTRIDE

    nc = Bacc(
        "TRN2",
        target_bir_lowering=False,
        debug=False,
        num_devices=cfg.NC,
        num_swdge_queues=4,
    )

    # I/O
    xT = nc.dram_tensor("xT", [F, NLOCP], f32, kind="ExternalInput")
    W1f = nc.dram_tensor("W1f", [F, HID], f32, kind="ExternalInput")
    W2f = nc.dram_tensor("W2f", [HID, HID], f32, kind="ExternalInput")
    Woutf = nc.dram_tensor("Woutf", [HID, ACTD], f32, kind="ExternalInput")
    b1col = nc.dram_tensor("b1col", [HID, 1], f32, kind="ExternalInput")
    b2col = nc.dram_tensor("b2col", [HID, 1], f32, kind="ExternalInput")
    boutcol = nc.dram_tensor("boutcol", [ACTD, 1], f32, kind="ExternalInput")
    dinvb = nc.dram_tensor("dinvb", [128, TILES], f32, kind="ExternalInput")
    dinvrep = nc.dram_tensor("dinvrep", [HID, NLOCP], f32, kind="ExternalInput")
    idx16 = nc.dram_tensor("idx16", [128, TOK // 16], i16, kind="ExternalInput")
    node16 = nc.dram_tensor("node16", [128, TOTCH], f32, kind="ExternalInput")
    iota16 = nc.dram_tensor("iota16", [128, SCOLS], f32, kind="ExternalInput")
    out_fm = nc.dram_tensor("out_fm", [ACTD, NLOCP], f32, kind="ExternalOutput")
    tbl_dbg = nc.dram_tensor("tbl_dbg", [VROWS, TS], bf16, kind="ExternalInput") if sub == 7 else None

    # internal DRAM: fat gather tables (local shard + allgathered full)
    tbl_loc = [nc.dram_tensor(f"tbl_loc{l}", [NLOCP, TS], bf16) for l in (1, 2)]
    tbl_full = [nc.dram_tensor(f"tbl_full{l}", [VROWS, TS], bf16) for l in (1, 2)]

    with tile.TileContext(nc) as tc:
        nc.gpsimd.load_library(library_config.mlp)
        from contextlib import ExitStack

        with ExitStack() as ctx:
            consts = ctx.enter_context(tc.tile_pool(name="consts", bufs=1))
            mpool = ctx.enter_context(tc.tile_pool(name="msgs", bufs=4))
            spool = ctx.enter_context(tc.tile_pool(name="sel", bufs=2))
            ipool = ctx.enter_context(tc.tile_pool(name="idxt", bufs=4))
            wkpool = ctx.enter_context(tc.tile_pool(name="work", bufs=2))
            pagg = ctx.enter_context(tc.tile_pool(name="pagg", bufs=2, space="PSUM"))
            ptbl = ctx.enter_context(tc.tile_pool(name="ptbl", bufs=2, space="PSUM"))
            pout = ctx.enter_context(tc.tile_pool(name="pout", bufs=2, space="PSUM"))

            # ---- resident constants
            w1_t = consts.tile([F, HID], f32)
            nc.sync.dma_start(w1_t[:], W1f[:])
            w2_t = consts.tile([HID, HID], f32)
            nc.sync.dma_start(w2_t[:], W2f[:])
            wout_t = consts.tile([HID, ACTD], f32)
            nc.sync.dma_start(wout_t[:], Woutf[:])
            b1_t = consts.tile([HID, 1], f32)
            nc.sync.dma_start(b1_t[:], b1col[:])
            b2_t = consts.tile([HID, 1], f32)
            nc.sync.dma_start(b2_t[:], b2col[:])
            bout_t = consts.tile([ACTD, 1], f32)
            nc.sync.dma_start(bout_t[:], boutcol[:])
            dinvb_t = consts.tile([128, TILES], f32)
            nc.sync.dma_start(dinvb_t[:], dinvb[:])
            dinvrep_t = consts.tile([HID, NLOCP], f32)
            nc.sync.dma_start(dinvrep_t[:], dinvrep[:])
            iota_t = consts.tile([128, SCOLS], f32)
            nc.sync.dma_start(iota_t[:], iota16[:])
            node_t = consts.tile([128, TOTCH], f32)
            nc.sync.dma_start(node_t[:], node16[:])
            xT_t = consts.tile([F, NLOCP], f32)
            nc.sync.dma_start(xT_t[:], xT[:])
            out1_t = consts.tile([HID, NLOCP], f32)

            def build_table(l, src_fm, kdim, w_t):
                """table_l = dinv * (src_fm.T @ W) ; src_fm [kdim, NLOCP]."""
                if sub == 8:
                    return
                BATCH = 16
                nb = -(-TILES // BATCH)
                for b in range(nb):
                    t0, t1 = b * BATCH, min((b + 1) * BATCH, TILES)
                    ps = ptbl.tile([128, BATCH, HID], f32, tag="ptbl")
                    for t in range(t0, t1):
                        nc.tensor.matmul(
                            ps[:, t - t0, :],
                            src_fm[:, t * 128 : (t + 1) * 128],
                            w_t[:],
                            start=True,
                            stop=True,
                        )
                    sb = wkpool.tile([128, BATCH, HID], bf16, tag="tblsb")
                    nc.vector.tensor_tensor(
                        sb[:, : t1 - t0, :],
                        ps[:, : t1 - t0, :],
                        dinvb_t[:, t0:t1, None].to_broadcast([128, t1 - t0, HID]),
                        op=Alu.mult,
                    )
                    # rows 128t+p of the fat local table, payload cols 0:HID
                    dst = tbl_loc[l][:, 0:HID].rearrange(
                        "(t p) d -> p t d", p=128
                    )[:, t0:t1, :]
                    nc.sync.dma_start(dst, sb[:, : t1 - t0, :])
                # allgather shards into the full fat table
                nc.gpsimd.collective_compute(
                    "AllGather",
                    Alu.bypass,
                    replica_groups=[list(range(cfg.NC))],
                    ins=[tbl_loc[l].ap().opt()],
                    outs=[tbl_full[l].ap().opt()],
                )

            def aggregate(l, out_t, bias_t):
                """out_t[:, :] = relu(dinv_dst * (S.T-reduced gathered msgs) + b)."""
                tblF = tbl_dbg if sub == 7 else tbl_full[l]
                ch_cursor = 0
                qrr = [0]
                for wi in range(NWIN):
                    ps = pagg.tile([HID, WIN], f32, tag="pagg")
                    wch0 = ch_cursor
                    # gather both halves' messages for this window
                    mtiles = {}
                    for h in (0, 1):
                        tok0, tok1 = meta["tok_range"][(wi, h)]
                        ntok = tok1 - tok0
                        if ntok == 0:
                            continue
                        mt = mpool.tile([128, ntok // 128, HID], bf16, tag=f"m{h}")
                        it = ipool.tile([128, ntok // 16], i16, tag=f"i{h}")
                        nc.sync.dma_start(it[:], idx16[:, tok0 // 16 : tok1 // 16])
                        if sub == 3:
                            nc.vector.memset(mt[:], 0.25)
                        if sub not in (2, 3):  # sub6 keeps gathers
                            emit_dma_gather(
                                nc.gpsimd,
                                mt[:],
                                tblF[h * HALF : (h + 1) * HALF, 0:HID],
                                it[:],
                                ntok,
                                HID,
                                TS,
                                queue_num=qrr[0] % 4,
                            )
                            qrr[0] += 1
                        mtiles[h] = mt
                    # one-hot S for all chunks of this window
                    wch = sum(int(NCH[wi, h, s]) for h in (0, 1) for s in range(NSW))
                    if sub == 5 and wch > 0:
                        st = spool.tile([128, wch, SCOLS], bf16, tag="sel")
                        nc.vector.tensor_tensor(
                            st[:, 0, :], node_t[:, 0:1].to_broadcast([128, SCOLS]),
                            iota_t[:], op=Alu.is_equal)
                        nc.vector.memset(st[:, 1:, :], 0.0)
                    if wch > 0 and sub not in (2, 5, 6):
                        st = spool.tile([128, wch, SCOLS], bf16, tag="sel")
                        nc.vector.tensor_tensor(
                            st[:],
                            node_t[:, wch0 : wch0 + wch, None].to_broadcast(
                                [128, wch, SCOLS]
                            ),
                            iota_t[:, None, :].to_broadcast([128, wch, SCOLS]),
                            op=Alu.is_equal,
                        )
                    # matmul-accumulate
                    if sub in (1, 2, 6):
                        nc.vector.memset(ps[:], 0.5)
                    first = True
                    chw = 0
                    for h in (() if sub in (1, 2, 6) else (0, 1)):
                        mh = 0
                        for s in range(NSW):
                            for _k in range(int(NCH[wi, h, s])):
                                is_last = chw == wch - 1
                                nc.tensor.matmul(
                                    ps[:, s * SCOLS : (s + 1) * SCOLS],
                                    mtiles[h][:, mh, :],
                                    st[:, chw, :],
                                    start=first,
                                    stop=is_last,
                                )
                                first = False
                                mh += 1
                                chw += 1
                    ch_cursor += wch
                    # evict window: relu(dinv * psum + b)
                    tmp = wkpool.tile([HID, WIN], f32, tag="evt")
                    nc.vector.tensor_tensor(
                        tmp[:],
                        ps[:],
                        dinvrep_t[:, wi * WIN : (wi + 1) * WIN],
                        op=Alu.mult,
                    )
                    nc.scalar.activation(
                        out_t[:, wi * WIN : (wi + 1) * WIN], tmp[:], Act.Relu,
                        bias=bias_t[:],
                    )

            # ---- layer 1
            build_table(0, xT_t[:], F, w1_t)
            if stage >= 2:
                aggregate(0, out1_t, b1_t)
            else:
                nc.vector.memset(out1_t[:], 0.125)
            # ---- layer 2
            if stage >= 3:
                build_table(1, out1_t[:], HID, w2_t)
            out2_t = consts.tile([HID, NLOCP], f32)
            if stage >= 4:
                aggregate(1, out2_t, b2_t)
            else:
                nc.vector.memset(out2_t[:], 0.25)
            # ---- output layer: logits_fm = Wout.T @ out2 + bout
            for wi in range(NWIN):
                ps = pout.tile([ACTD, WIN], f32, tag="pl")
                nc.tensor.matmul(
                    ps[:],
                    wout_t[:],
                    out2_t[:, wi * WIN : (wi + 1) * WIN],
                    start=True,
                    stop=True,
                )
                lsb = wkpool.tile([ACTD, WIN], f32, tag="lsb")
                nc.vector.tensor_scalar_add(lsb[:], ps[:], bout_t[:])
                nc.sync.dma_start(out_fm[:, wi * WIN : (wi + 1) * WIN], lsb[:])

    nc.compile()
    return nc


# ----------------------------------------------------------------------------
# Entry point
# ----------------------------------------------------------------------------
_CACHE = {}


def run(x, edge_index, W1, b1, W2, b2, Wout, bout, cfg, trace=False):
    from concourse import bass_utils

    in_maps, meta = preprocess(x, edge_index, W1, b1, W2, b2, Wout, bout, cfg)
    key = (cfg.N, cfg.E, meta["TOK"], tuple(meta["NCH"].ravel().tolist()))
    import os
    stage = int(os.environ.get('K_STAGE', '99'))
    sub = int(os.environ.get('K_SUB', '0'))
    key = key + (stage, sub)
    if key not in _CACHE:
        _CACHE[key] = build(meta, cfg, stage, sub)
    nc = _CACHE[key]
    res = bass_utils.run_bass_kernel_spmd(
        nc, in_maps, core_ids=list(range(cfg.NC)), trace=trace
    )
    out = np.empty((cfg.N, cfg.ACT), np.float32)
    for c in range(cfg.NC):
        out[c * cfg.NLOC : (c + 1) * cfg.NLOC] = (
            res.results[c]["out_fm"][:, : cfg.NLOC].T
        )
    return out, res


def kernel(x, edge_index, W1, b1, W2, b2, Wout, bout):
    out, _ = run(x, edge_index, W1, b1, W2, b2, Wout, bout, CFG_FULL)
    return out



# revision 12
# speedup vs baseline: 2.7490x; 1.0371x over previous
"""2-layer GCN (PyG GCNConv semantics) on 8 Trainium2 NeuronCores.

Strategy (dst-sharded, per core):
  - Host: add self-loops, compute symmetric norm dinv = 1/sqrt(deg),
    sort each core's incoming edges by (window, src-half, sub-window),
    pad each (w,h,sw) group's slot count to a cross-core-uniform
    multiple of 128 so one SPMD program serves all 8 cores.
  - Device, per GCN layer:
      table[v] = dinv[v] * (h @ W)           (node-major bf16 rows, 256B stride)
      AllGather table shards
      per 512-node window: dma_gather 64B message rows (token slots),
      build one-hot S via DVE is_equal(nodeof, iota), PE matmuls
      psum[:, 64*sw:64*sw+64] += M_chunk.T @ S_chunk  (feature-major psum),
      evict: out = relu(dinv_dst * psum + b) on DVE+ACT.
  - Final: logits = Wout.T @ out2 (feature-major), host transposes.
"""

import sys
import numpy as np

sys.path.insert(0, "/opt/trn_rl_repo")

import ml_dtypes  # noqa: E402

BF16 = ml_dtypes.bfloat16


# ----------------------------------------------------------------------------
# Config
# ----------------------------------------------------------------------------
class Cfg:
    def __init__(self, N, E, F, HID, ACT, NC, WIN, SCOLS):
        self.N, self.E, self.F, self.HID, self.ACT, self.NC = N, E, F, HID, ACT, NC
        self.WIN, self.SCOLS = WIN, SCOLS
        self.NLOC = N // NC                       # real nodes per core
        assert self.NLOC * NC == N
        self.NWIN = -(-self.NLOC // WIN)          # windows per core
        self.NLOCP = self.NWIN * WIN              # padded nodes per core
        assert self.NLOCP % 128 == 0
        self.NSW = WIN // SCOLS                   # sub-windows per window
        self.TILES = self.NLOCP // 128            # 128-node tiles per core
        self.VROWS = NC * self.NLOCP              # padded global rows
        self.HALF = self.VROWS // 2               # rows per src half
        assert self.HALF <= 32767, "int16 gather index overflow"
        assert self.HALF % self.NLOCP == 0, "half boundary must align to core shards"
        self.TSTRIDE = 128                        # bf16 elements per table row (256B)


CFG_FULL = Cfg(N=50000, E=1600000, F=128, HID=32, ACT=64, NC=8, WIN=512, SCOLS=64)
# 50000/8 = 6250 -> NWIN=13, NLOCP=6656, VROWS=53248, HALF=26624


# ----------------------------------------------------------------------------
# Host preprocessing
# ----------------------------------------------------------------------------
def preprocess(x, edge_index, W1, b1, W2, b2, Wout, bout, cfg):
    """Returns (in_maps, meta). meta drives program structure and must be
    identical for every core (it is: NCH is maxed across cores)."""
    N, NC, NLOC, NLOCP = cfg.N, cfg.NC, cfg.NLOC, cfg.NLOCP
    NWIN, WIN, NSW, SCOLS = cfg.NWIN, cfg.WIN, cfg.NSW, cfg.SCOLS

    src = np.asarray(edge_index[0], dtype=np.int64)
    dst = np.asarray(edge_index[1], dtype=np.int64)
    loop = np.arange(N, dtype=np.int64)
    src = np.concatenate([src, loop])
    dst = np.concatenate([dst, loop])
    M = src.shape[0]

    deg = np.bincount(dst, minlength=N).astype(np.float64)
    dinv = np.where(deg > 0, 1.0 / np.sqrt(deg), 0.0).astype(np.float32)

    core = dst // NLOC
    local = dst - core * NLOC
    w = local // WIN
    sw = (local % WIN) // SCOLS
    nodeof = (local % SCOLS).astype(np.float32)

    csrc = src // NLOC
    vv = csrc * NLOCP + (src - csrc * NLOC)       # padded virtual row of src
    half = vv // cfg.HALF
    idxval = (vv - half * cfg.HALF).astype(np.int16)

    NKEY_PER_CORE = NWIN * 2 * NSW
    key = ((core * NWIN + w) * 2 + half) * NSW + sw
    cnt = np.bincount(key, minlength=NC * NKEY_PER_CORE).reshape(NC, NWIN, 2, NSW)
    NCH = -(-cnt // 128)                          # ceil
    NCH = NCH.max(axis=0)                         # [NWIN, 2, NSW] cross-core program
    slots_per_key = (NCH * 128).ravel()           # flat (w,h,sw)
    slot_base = np.concatenate([[0], np.cumsum(slots_per_key)])
    TOK = int(slot_base[-1])
    TOTCH = TOK // 128

    order = np.argsort(key, kind="stable")
    ks = key[order]
    gstart = np.concatenate([[0], np.cumsum(cnt.ravel())])[:-1]
    pos = np.arange(M) - gstart[ks]
    kk = ks % NKEY_PER_CORE
    slot = slot_base[kk] + pos
    oc = ks // NKEY_PER_CORE

    idxarr = np.zeros((NC, TOK), np.int16)        # pad slots gather row 0
    nodearr = np.full((NC, TOK), -1, np.float32)  # pad slots match no column
    idxarr[oc, slot] = idxval[order]
    nodearr[oc, slot] = nodeof[order]

    # per-(w,h) token ranges and chunk ranges
    tok_of = slot_base.reshape(-1)                # len NKEY+1 over (w,h,sw)
    meta = {
        "NCH": NCH,
        "TOK": TOK,
        "TOTCH": TOTCH,
        "tok_range": {},                          # (w,h) -> (tok0, tok1)
    }
    for wi in range(NWIN):
        for h in range(2):
            k0 = (wi * 2 + h) * NSW
            meta["tok_range"][(wi, h)] = (int(tok_of[k0]), int(tok_of[k0 + NSW]))

    # per-core inputs
    x = np.asarray(x, np.float32)
    dinv_pad = np.zeros(NC * NLOCP, np.float32)
    for c in range(NC):
        dinv_pad[c * NLOCP : c * NLOCP + NLOC] = dinv[c * NLOC : (c + 1) * NLOC]

    in_maps = []
    iota = np.tile(np.arange(SCOLS, dtype=np.float32)[None, :], (128, 1)).astype(BF16)
    for c in range(NC):
        xc = np.zeros((cfg.F, NLOCP), np.float32)
        xc[:, :NLOC] = x[c * NLOC : (c + 1) * NLOC].T
        dl = dinv_pad[c * NLOCP : (c + 1) * NLOCP]
        idx_rep = np.tile(idxarr[c].reshape(TOK // 16, 16).T, (8, 1))
        node_t = nodearr[c].reshape(TOTCH, 128).T.astype(BF16)
        in_maps.append(
            {
                "xT": xc.copy(),
                "W1f": np.asarray(W1, np.float32),
                "W2f": np.asarray(W2, np.float32),
                "Woutf": np.asarray(Wout, np.float32),
                "b1col": np.asarray(b1, np.float32).reshape(cfg.HID, 1),
                "b2col": np.asarray(b2, np.float32).reshape(cfg.HID, 1),
                "boutcol": np.asarray(bout, np.float32).reshape(cfg.ACT, 1),
                "dinvb": dl.reshape(cfg.TILES, 128).T.copy(),      # [128, TILES]
                "dinvrep": np.tile(dl[None, :], (cfg.HID, 1)).astype(BF16),
                "idx16": np.ascontiguousarray(idx_rep),            # [128, TOK/16]
                "node16": np.ascontiguousarray(node_t),            # [128, TOTCH] bf16
                "iota16": iota,                                    # [128, SCOLS]
            }
        )
    return in_maps, meta


# ----------------------------------------------------------------------------
# Device program
# ----------------------------------------------------------------------------
def emit_dma_gather(gp, out_ap, in_ap, idxs_ap, num_idxs, elem_size, elem_step,
                    queue_num=0):
    """bass.dma_gather (non-transpose, DRAM src) without the 256B elem assert
    (the 256B constraint applies to the row *stride*, encoded below)."""
    import concourse.mybir as mybir
    from concourse import ap_utils
    from concourse.bass import round_up_to_multiple, exact_div

    self = gp
    assert idxs_ap.dtype == mybir.dt.int16
    assert in_ap.dtype == out_ap.dtype
    assert ap_utils.ap_is_contiguous(in_ap.ap[1:])
    assert ap_utils.ap_is_contiguous(out_ap.ap[1:])
    assert ap_utils.ap_is_contiguous(idxs_ap.ap[1:])
    assert in_ap.ap[-1][1] == out_ap.ap[-1][1] == elem_size
    assert out_ap.ap[0][1] * out_ap.ap[1][1] == round_up_to_multiple(num_idxs, 128)
    assert in_ap.ap[0][0] == elem_step
    stride_bytes = elem_step * mybir.dt.size(in_ap.dtype)
    stride_bytes_256 = exact_div(stride_bytes, 256)
    assert stride_bytes_256 < 256
    _in_ap = self.lower_ap_dma(in_ap, for_custom_bir_dma=True)
    _idxs_ap = self.lower_ap(idxs_ap)
    _out_ap = self.lower_ap(out_ap)
    return self.add_instruction(
        mybir.InstDMAGatherAnt(
            name=self.bass.get_next_instruction_name(),
            ins=[*_in_ap, _idxs_ap, self.lower_val_access(self.to_reg(num_idxs))],
            outs=[_out_ap],
            transpose=False,
            num_idxs=num_idxs,
            elem_size=elem_size,
            stride_bytes_256=stride_bytes_256,
            gen_mode=0,
            single_packet=False,
            queue_num=queue_num,
            sbuf_tokens_per_rank=0,
            sbuf_free_dim_per_rank=0,
            sbuf_free_dim_pad_per_rank=0,
            sbuf_byte_offset=0,
        )
    )


def build(meta, cfg, stage=99, sub=0):
    import concourse.mybir as mybir
    import concourse.tile as tile
    from concourse.bacc import Bacc
    from concourse import library_config
    from concourse.tile_rust import add_dep_helper

    f32, bf16, i16 = mybir.dt.float32, mybir.dt.bfloat16, mybir.dt.int16
    Alu = mybir.AluOpType
    Act = mybir.ActivationFunctionType
    NCH, TOK, TOTCH = meta["NCH"], meta["TOK"], meta["TOTCH"]
    NWIN, WIN, NSW, SCOLS = cfg.NWIN, cfg.WIN, cfg.NSW, cfg.SCOLS
    HID, F, ACTD, TILES = cfg.HID, cfg.F, cfg.ACT, cfg.TILES
    NLOCP, VROWS, HALF, TS = cfg.NLOCP, cfg.VROWS, cfg.HALF, cfg.TSTRIDE

    nc = Bacc(
        "TRN2",
        target_bir_lowering=False,
        debug=False,
        num_devices=cfg.NC,
        num_swdge_queues=4,
    )

    # I/O
    xT = nc.dram_tensor("xT", [F, NLOCP], f32, kind="ExternalInput")
    W1f = nc.dram_tensor("W1f", [F, HID], f32, kind="ExternalInput")
    W2f = nc.dram_tensor("W2f", [HID, HID], f32, kind="ExternalInput")
    Woutf = nc.dram_tensor("Woutf", [HID, ACTD], f32, kind="ExternalInput")
    b1col = nc.dram_tensor("b1col", [HID, 1], f32, kind="ExternalInput")
    b2col = nc.dram_tensor("b2col", [HID, 1], f32, kind="ExternalInput")
    boutcol = nc.dram_tensor("boutcol", [ACTD, 1], f32, kind="ExternalInput")
    dinvb = nc.dram_tensor("dinvb", [128, TILES], f32, kind="ExternalInput")
    dinvrep = nc.dram_tensor("dinvrep", [HID, NLOCP], bf16, kind="ExternalInput")
    idx16 = nc.dram_tensor("idx16", [128, TOK // 16], i16, kind="ExternalInput")
    node16 = nc.dram_tensor("node16", [128, TOTCH], bf16, kind="ExternalInput")
    iota16 = nc.dram_tensor("iota16", [128, SCOLS], bf16, kind="ExternalInput")
    out_fm = nc.dram_tensor("out_fm", [ACTD, NLOCP], f32, kind="ExternalOutput")
    tbl_dbg = nc.dram_tensor("tbl_dbg", [VROWS, TS], bf16, kind="ExternalInput") if sub == 7 else None

    # internal DRAM: fat gather tables (local shard + allgathered full)
    tbl_loc = [nc.dram_tensor(f"tbl_loc{l}", [NLOCP, TS], bf16) for l in (1, 2)]
    tbl_full = [nc.dram_tensor(f"tbl_full{l}", [VROWS, TS], bf16) for l in (1, 2)]

    with tile.TileContext(nc) as tc:
        nc.gpsimd.load_library(library_config.mlp)
        from contextlib import ExitStack

        with ExitStack() as ctx:
            consts = ctx.enter_context(tc.tile_pool(name="consts", bufs=1))
            wkpool = ctx.enter_context(tc.tile_pool(name="work", bufs=2))
            pagg = ctx.enter_context(tc.tile_pool(name="pagg", bufs=4, space="PSUM"))
            ptbl = ctx.enter_context(tc.tile_pool(name="ptbl", bufs=2, space="PSUM"))
            pout = ctx.enter_context(tc.tile_pool(name="pout", bufs=2, space="PSUM"))

            # ---- resident constants
            w1_t = consts.tile([F, HID], f32)
            nc.sync.dma_start(w1_t[:], W1f[:])
            w2_t = consts.tile([HID, HID], f32)
            nc.sync.dma_start(w2_t[:], W2f[:])
            wout_t = consts.tile([HID, ACTD], f32)
            nc.sync.dma_start(wout_t[:], Woutf[:])
            b1_t = consts.tile([HID, 1], f32)
            nc.sync.dma_start(b1_t[:], b1col[:])
            b2_t = consts.tile([HID, 1], f32)
            nc.sync.dma_start(b2_t[:], b2col[:])
            bout_t = consts.tile([ACTD, 1], f32)
            nc.sync.dma_start(bout_t[:], boutcol[:])
            dinvb_t = consts.tile([128, TILES], f32)
            nc.sync.dma_start(dinvb_t[:], dinvb[:])
            dinvrep_t = consts.tile([HID, NLOCP], bf16)
            nc.sync.dma_start(dinvrep_t[:], dinvrep[:])
            iota_t = consts.tile([128, SCOLS], bf16)
            nc.sync.dma_start(iota_t[:], iota16[:])
            node_t = consts.tile([128, TOTCH], bf16)
            nc.sync.dma_start(node_t[:], node16[:])
            out1_t = consts.tile([HID, NLOCP], f32)

            def build_table(l, src_fm, kdim, w_t):
                """table_l = dinv * (src_fm.T @ W) ; src_fm [kdim, NLOCP]."""
                if sub == 8:
                    return
                BATCH = 16
                nb = -(-TILES // BATCH)
                for b in range(nb):
                    t0, t1 = b * BATCH, min((b + 1) * BATCH, TILES)
                    ps = ptbl.tile([128, BATCH, HID], f32, tag="ptbl")
                    for t in range(t0, t1):
                        nc.tensor.matmul(
                            ps[:, t - t0, :],
                            src_fm[:, t * 128 : (t + 1) * 128],
                            w_t[:],
                            start=True,
                            stop=True,
                        )
                    sb = wkpool.tile([128, BATCH, HID], bf16, tag="tblsb")
                    nc.vector.tensor_tensor(
                        sb[:, : t1 - t0, :],
                        ps[:, : t1 - t0, :],
                        dinvb_t[:, t0:t1, None].to_broadcast([128, t1 - t0, HID]),
                        op=Alu.mult,
                    )
                    # rows 128t+p of the fat local table, payload cols 0:HID
                    dst = tbl_loc[l][:, 0:HID].rearrange(
                        "(t p) d -> p t d", p=128
                    )[:, t0:t1, :]
                    nc.sync.dma_start(dst, sb[:, : t1 - t0, :])
                # allgather shards into the full fat table
                nc.gpsimd.collective_compute(
                    "AllGather",
                    Alu.bypass,
                    replica_groups=[list(range(cfg.NC))],
                    ins=[tbl_loc[l].ap().opt()],
                    outs=[tbl_full[l].ap().opt()],
                )

            def aggregate(l, out_t, bias_t):
                """out_t[:, :] = relu(dinv_dst * (S.T-reduced gathered msgs) + b)."""
                tblF = tbl_dbg if sub == 7 else tbl_full[l]
                ch_cursor = 0
                qrr = [0]
                for wi in range(NWIN):
                    ps = pagg.tile([HID, WIN], f32, tag="pagg")
                    wch0 = ch_cursor
                    # gather both halves' messages for this window
                    mtiles = {}
                    for h in (0, 1):
                        tok0, tok1 = meta["tok_range"][(wi, h)]
                        ntok = tok1 - tok0
                        if ntok == 0:
                            continue
                        mt = mpool.tile([128, ntok // 128, HID], bf16, tag=f"m{h}")
                        it = ipool.tile([128, ntok // 16], i16, tag=f"i{h}")
                        nc.sync.dma_start(it[:], idx16[:, tok0 // 16 : tok1 // 16])
                        if sub == 3:
                            nc.vector.memset(mt[:], 0.25)
                        if sub not in (2, 3):  # sub6 keeps gathers
                            emit_dma_gather(
                                nc.gpsimd,
                                mt[:],
                                tblF[h * HALF : (h + 1) * HALF, 0:HID],
                                it[:],
                                ntok,
                                HID,
                                TS,
                                queue_num=qrr[0] % 4,
                            )
                            qrr[0] += 1
                        mtiles[h] = mt
                    # one-hot S for all chunks of this window
                    wch = sum(int(NCH[wi, h, s]) for h in (0, 1) for s in range(NSW))
                    if sub == 5 and wch > 0:
                        st = spool.tile([128, wch, SCOLS], bf16, tag="sel")
                        nc.vector.tensor_tensor(
                            st[:, 0, :], node_t[:, 0:1].to_broadcast([128, SCOLS]),
                            iota_t[:], op=Alu.is_equal)
                        nc.vector.memset(st[:, 1:, :], 0.0)
                    if wch > 0 and sub not in (2, 5, 6):
                        st = spool.tile([128, wch, SCOLS], bf16, tag="sel")
                        nc.vector.tensor_tensor(
                            st[:],
                            node_t[:, wch0 : wch0 + wch, None].to_broadcast(
                                [128, wch, SCOLS]
                            ),
                            iota_t[:, None, :].to_broadcast([128, wch, SCOLS]),
                            op=Alu.is_equal,
                        )
                    # matmul-accumulate
                    if sub in (1, 2, 6):
                        nc.vector.memset(ps[:], 0.5)
                    first = True
                    chw = 0
                    for h in (() if sub in (1, 2, 6) else (0, 1)):
                        mh = 0
                        for s in range(NSW):
                            for _k in range(int(NCH[wi, h, s])):
                                is_last = chw == wch - 1
                                nc.tensor.matmul(
                                    ps[:, s * SCOLS : (s + 1) * SCOLS],
                                    mtiles[h][:, mh, :],
                                    st[:, chw, :],
                                    start=first,
                                    stop=is_last,
                                )
                                first = False
                                mh += 1
                                chw += 1
                    ch_cursor += wch
                    # evict window: relu(dinv * psum + b)
                    tmp = wkpool.tile([HID, WIN], f32, tag="evt")
                    nc.vector.tensor_tensor(
                        tmp[:],
                        ps[:],
                        dinvrep_t[:, wi * WIN : (wi + 1) * WIN],
                        op=Alu.mult,
                    )
                    nc.scalar.activation(
                        out_t[:, wi * WIN : (wi + 1) * WIN], tmp[:], Act.Relu,
                        bias=bias_t[:],
                    )

            # ---- layer 1
            with tc.tile_pool(name="xt", bufs=1) as xtpool:
                xT_t = xtpool.tile([F, NLOCP], f32)
                nc.sync.dma_start(xT_t[:], xT[:])
                build_table(0, xT_t[:], F, w1_t)
            mpool = ctx.enter_context(tc.tile_pool(name="msgs", bufs=8))
            spool = ctx.enter_context(tc.tile_pool(name="sel", bufs=2))
            ipool = ctx.enter_context(tc.tile_pool(name="idxt", bufs=7))
            if stage >= 2:
                aggregate(0, out1_t, b1_t)
            else:
                nc.vector.memset(out1_t[:], 0.125)
            # ---- layer 2
            if stage >= 3:
                build_table(1, out1_t[:], HID, w2_t)
            out2_t = consts.tile([HID, NLOCP], f32)
            if stage >= 4:
                aggregate(1, out2_t, b2_t)
            else:
                nc.vector.memset(out2_t[:], 0.25)
            # ---- output layer: logits_fm = Wout.T @ out2 + bout
            for wi in range(NWIN):
                ps = pout.tile([ACTD, WIN], f32, tag="pl")
                nc.tensor.matmul(
                    ps[:],
                    wout_t[:],
                    out2_t[:, wi * WIN : (wi + 1) * WIN],
                    start=True,
                    stop=True,
                )
                lsb = wkpool.tile([ACTD, WIN], f32, tag="lsb")
                nc.vector.tensor_scalar_add(lsb[:], ps[:], bout_t[:])
                nc.sync.dma_start(out_fm[:, wi * WIN : (wi + 1) * WIN], lsb[:])

    nc.compile()
    return nc


# ----------------------------------------------------------------------------
# Entry point
# ----------------------------------------------------------------------------
_CACHE = {}


def run(x, edge_index, W1, b1, W2, b2, Wout, bout, cfg, trace=False):
    from concourse import bass_utils

    in_maps, meta = preprocess(x, edge_index, W1, b1, W2, b2, Wout, bout, cfg)
    key = (cfg.N, cfg.E, meta["TOK"], tuple(meta["NCH"].ravel().tolist()))
    import os
    stage = int(os.environ.get('K_STAGE', '99'))
    sub = int(os.environ.get('K_SUB', '0'))
    key = key + (stage, sub)
    if key not in _CACHE:
        _CACHE[key] = build(meta, cfg, stage, sub)
    nc = _CACHE[key]
    res = bass_utils.run_bass_kernel_spmd(
        nc, in_maps, core_ids=list(range(cfg.NC)), trace=trace
    )
    out = np.empty((cfg.N, cfg.ACT), np.float32)
    for c in range(cfg.NC):
        out[c * cfg.NLOC : (c + 1) * cfg.NLOC] = (
            res.results[c]["out_fm"][:, : cfg.NLOC].T
        )
    return out, res


def kernel(x, edge_index, W1, b1, W2, b2, Wout, bout):
    out, _ = run(x, edge_index, W1, b1, W2, b2, Wout, bout, CFG_FULL)
    return out

